# revision 1
# baseline (speedup 1.0000x reference)
"""Trainium2 Bass kernel for DynamicHyperedgeWeightLearner.

Strategy (8 NeuronCores, SPMD single NEFF), v2 — upload-minimal:
  - The graded time is dominated by host->device input bytes, so the
    dense H slab (8 MB/core) and [x|x^2] (4 MB/core) uploads of v1 are
    gone.  Each core receives only: x in bf16 (2 MB), an int16 member-
    index table (~30 KB), per-edge 1/deg and pad-correction scalars,
    and the tiny MLP weights.
  - Hyperedges (M=4096) are degree-sorted and dealt into 8 cores x
    4 groups x 128 edges with a shared per-group pad K (group max
    degree), so every core sees identical shapes (one NEFF for all).
  - Per core, dma_gather (non-transpose; transpose mode faults on this
    HW path; SWDGE ring is hard-capped at 1024 descriptors/call) lands
    member rows edge-major: partition = edge, free axis = member x dim.
    One gather feeds everything:
      max/min  -> DVE reduce over the member axis (pad = last member),
                  then PE transpose to feature-major.
      sum/sumsq-> accumulated directly in transposed (feature-major)
                  form on PE as K matmuls against a bf16 identity;
                  squares from ScalarE (bf16); the pad contribution
                  (K-d)*x_last is subtracted using broadcast per-edge
                  coefficient rows.  Empty edges are padded with node 0
                  and coef K, making all stats 0.
  - The tiny MLP runs feature-major on PE; t_embed is folded into an
    effective b1 on the host (constant across hyperedges).

bf16 note: x quantization changes the final sigmoids by ~1e-4 relative,
far below the 2e-2 gate.
"""

import numpy as np
import ml_dtypes

N, M, D = 8192, 4096, 128
P = 128
C = 8                 # cores
GROUPS = 4            # degree groups
EPC = P               # edges per core per group
MC = GROUPS * EPC     # 512 edges per core
GSZ = C * EPC         # 1024 global edges per group
T_DIM, MLP_H = 32, 64
RING = 1024           # SWDGE descriptor-ring capacity (scratch/16)
CH = RING // P        # member columns per gather call
NQ = 4                # SWDGE queues (ucode max)

last_run_info = {}


def _iw(kg):
    return 8 * sum(kg)


def _prep(node_embeddings, incidence_matrix, time_step,
          W_t, b_t, W1, b1, W2, b2, W3, b3):
    """Host-side preprocessing -> per-core input maps + assembly info."""
    bf16 = ml_dtypes.bfloat16
    x = np.ascontiguousarray(node_embeddings, dtype=np.float32)
    H = np.ascontiguousarray(incidence_matrix, dtype=np.float32)

    nodes, edges = np.nonzero(H)
    order = np.argsort(edges, kind="stable")
    n_sorted = nodes[order].astype(np.int32)
    deg = np.bincount(edges, minlength=M).astype(np.int64)
    offs = np.zeros(M + 1, np.int64)
    np.cumsum(deg, out=offs[1:])

    rank = np.argsort(-deg, kind="stable")      # edge ids by degree desc
    kg = [int(max(1, deg[rank[g * GSZ:(g + 1) * GSZ]].max()))
          for g in range(GROUPS)]

    xg = np.ascontiguousarray(x).astype(bf16)               # (N, D)
    NS = N // C                                             # x shard rows

    # fp32 scalar chain identical to the reference
    t = np.float32(np.asarray(time_step, dtype=np.float32).reshape(()))
    t_embed = np.maximum(
        (t * np.asarray(W_t, np.float32)[:, 0] + np.asarray(b_t, np.float32)),
        np.float32(0.0)).astype(np.float32)
    W1 = np.asarray(W1, np.float32)
    b1_eff = (np.asarray(b1, np.float32)
              + W1[:, 3 * D:] @ t_embed).astype(np.float32)
    w1T = np.ascontiguousarray(W1[:, :3 * D].T)                   # (384, 64)
    w1T_dev = np.ascontiguousarray(
        w1T.reshape(3, P, MLP_H).transpose(1, 0, 2)).astype(bf16)  # (128, 3, 64)
    w2T = np.ascontiguousarray(np.asarray(W2, np.float32).T)      # (64, 32)
    w3T = np.ascontiguousarray(np.asarray(W3, np.float32).T)      # (32, 1)
    b1_dev = b1_eff.reshape(MLP_H, 1).copy()
    b2 = np.asarray(b2, np.float32).reshape(32, 1).copy()
    b3 = np.asarray(b3, np.float32).reshape(1, 1).copy()

    in_maps, eids = [], []
    for c in range(C):
        e = np.concatenate(
            [rank[g * GSZ + c * EPC: g * GSZ + (c + 1) * EPC]
             for g in range(GROUPS)])
        eids.append(e)
        cols = []
        coefP = np.zeros((P, GROUPS), np.float32)
        rdegP = np.zeros((P, GROUPS), np.float32)
        for g in range(GROUPS):
            K = kg[g]
            Jm = np.zeros((K, P), np.int16)
            for p in range(P):
                eid = e[g * EPC + p]
                d0 = int(deg[eid])
                if d0 > 0:
                    mem = n_sorted[offs[eid]:offs[eid] + d0]
                    Jm[:d0, p] = mem
                    Jm[d0:, p] = mem[-1]
                coefP[p, g] = K - d0
                rdegP[p, g] = 1.0 / max(d0, 1)
            lin = Jm.reshape(-1)
            cols.append(lin.reshape(-1, 16).T.copy())     # (16, K*8)
        idx16 = np.ascontiguousarray(np.concatenate(cols, axis=1), np.int16)
        coef = np.ascontiguousarray(coefP.T.reshape(1, MC))
        rdeg = np.ascontiguousarray(rdegP.T.reshape(1, MC))
        in_maps.append({
            "xg": np.ascontiguousarray(xg[c * NS:(c + 1) * NS]),
            "idx16": idx16, "coef": coef, "rdeg": rdeg,
            "w1T": w1T_dev, "b1": b1_dev,
            "w2T": w2T, "b2": b2, "w3T": w3T, "b3": b3,
        })
    return in_maps, eids, kg


def _build(kg, loops=1, ag=True):
    """Build the SPMD Bass program (one NEFF, all 8 cores).

    ag=False builds a single-core variant with a full (unsharded) x input
    and no collective — used only for local TimelineSim profiling."""
    import concourse.mybir as mybir
    import concourse.tile as tile
    from concourse import bacc
    from concourse.masks import make_identity

    f32 = mybir.dt.float32
    bf = mybir.dt.bfloat16
    iw = _iw(kg)

    nc = bacc.Bacc("TRN2", num_devices=C if ag else None)
    xg_d = nc.dram_tensor("xg", [N // C if ag else N, D], bf,
                          kind="ExternalInput")
    idx_d = nc.dram_tensor("idx16", [16, iw], mybir.dt.int16,
                           kind="ExternalInput")
    coef_d = nc.dram_tensor("coef", [1, MC], f32, kind="ExternalInput")
    rdeg_d = nc.dram_tensor("rdeg", [1, MC], f32, kind="ExternalInput")
    w1T_d = nc.dram_tensor("w1T", [P, 3, MLP_H], bf, kind="ExternalInput")
    b1_d = nc.dram_tensor("b1", [MLP_H, 1], f32, kind="ExternalInput")
    w2T_d = nc.dram_tensor("w2T", [MLP_H, 32], f32, kind="ExternalInput")
    b2_d = nc.dram_tensor("b2", [32, 1], f32, kind="ExternalInput")
    w3T_d = nc.dram_tensor("w3T", [32, 1], f32, kind="ExternalInput")
    b3_d = nc.dram_tensor("b3", [1, 1], f32, kind="ExternalInput")
    out_d = nc.dram_tensor("out", [1, MC], f32, kind="ExternalOutput")

    with tile.TileContext(nc) as tc:
        with (
            tc.tile_pool(name="dram", bufs=1, space="DRAM") as dram,
            tc.tile_pool(name="singles", bufs=1) as singles,
            tc.tile_pool(name="gpool", bufs=3) as gpool,
            tc.tile_pool(name="sqpool", bufs=3) as sqpool,
            tc.tile_pool(name="stage", bufs=4) as stage,
            tc.tile_pool(name="tmps", bufs=2) as tmps,
            tc.tile_pool(name="mlp", bufs=1) as mlp,
            tc.tile_pool(name="ps_st", bufs=1, space="PSUM") as ps_st,
            tc.tile_pool(name="ps_tr", bufs=2, space="PSUM") as ps_tr,
            tc.tile_pool(name="ps_mlp", bufs=1, space="PSUM") as ps_mlp,
        ):
            # ---- x shards -> full x via on-device AllGather ----
            if ag:
                ag_in = dram.tile([N // C, D], bf)
                xfull = dram.tile([N, D], bf, addr_space="Shared")
                nc.gpsimd.dma_start(ag_in[:, :], xg_d[:, :])
                nc.gpsimd.collective_compute(
                    "AllGather", mybir.AluOpType.bypass,
                    replica_groups=[list(range(C))],
                    ins=[ag_in[:, :].opt()], outs=[xfull[:, :].opt()],
                )
            else:
                xfull = xg_d

            # ---- resident loads ----
            idx_sb = singles.tile([P, iw], mybir.dt.int16)
            for r in range(8):
                nc.sync.dma_start(idx_sb[16 * r:16 * (r + 1), :], idx_d[:, :])
            coef_bc = singles.tile([P, MC], f32)
            nc.gpsimd.dma_start(coef_bc, coef_d[0:1, :].to_broadcast((P, MC)))
            rdeg_bc = singles.tile([P, MC], f32)
            nc.gpsimd.dma_start(rdeg_bc, rdeg_d[0:1, :].to_broadcast((P, MC)))
            w1T_sb = singles.tile([P, 3, MLP_H], bf)
            nc.sync.dma_start(w1T_sb, w1T_d[:, :, :])
            b1_sb = singles.tile([MLP_H, 1], f32)
            nc.sync.dma_start(b1_sb, b1_d[:, :])
            w2T_sb = singles.tile([MLP_H, 32], f32)
            nc.sync.dma_start(w2T_sb, w2T_d[:, :])
            b2_sb = singles.tile([32, 1], f32)
            nc.sync.dma_start(b2_sb, b2_d[:, :])
            w3T_sb = singles.tile([32, 1], f32)
            nc.sync.dma_start(w3T_sb, w3T_d[:, :])
            b3_sb = singles.tile([1, 1], f32)
            nc.sync.dma_start(b3_sb, b3_d[:, :])

            ident = singles.tile([P, P], f32)
            make_identity(nc, ident)
            identb = singles.tile([P, P], bf)
            make_identity(nc, identb)

            Kmax = max(kg)
            import os as _os
            _no_gather = bool(int(_os.environ.get("TIME_NO_GATHER", "0")))

            pre = None
            if _no_gather:
                pre = []
                offp = 0
                for g in range(GROUPS):
                    K = kg[g]
                    gf = singles.tile([P, Kmax, D], bf, name=f"pregth{g}")
                    for j0 in range(0, K, CH):
                        ch = min(CH, K - j0)
                        nc.gpsimd.dma_gather(
                            gf[:, j0:j0 + ch, :], xfull[:, :],
                            idx_sb[:, offp + 8 * j0:offp + 8 * (j0 + ch)],
                            ch * P, ch * P, D,
                        )
                    offp += 8 * K
                    pre.append(gf)

            for _rep in range(loops):
                maxT = mlp.tile([P, MC], f32, tag="maxT", name="maxT")
                minT = mlp.tile([P, MC], f32, tag="minT", name="minT")
                sT = mlp.tile([P, MC], f32, tag="sT", name="sT")
                qT = mlp.tile([P, MC], f32, tag="qT", name="qT")
                mu = mlp.tile([P, MC], f32, tag="mu", name="mu")
                sig = mlp.tile([P, MC], f32, tag="sig", name="sig")

                off = 0
                for g in range(GROUPS):
                    K = kg[g]
                    cs = slice(g * P, (g + 1) * P)
                    if _no_gather:
                        gfull = pre[g]
                    else:
                        gfull = gpool.tile([P, Kmax, D], bf, name="gth")
                    gth = gfull[:, 0:K, :]
                    if not _no_gather:
                        for j0 in range(0, K, CH):
                            ch = min(CH, K - j0)
                            nc.gpsimd.dma_gather(
                                gth[:, j0:j0 + ch, :], xfull[:, :],
                                idx_sb[:, off + 8 * j0:off + 8 * (j0 + ch)],
                                ch * P, ch * P, D,
                            )
                    off += 8 * K
                    sfull = sqpool.tile([P, Kmax, D], bf, name="sq")
                    sqt = sfull[:, 0:K, :]
                    nc.scalar.square(
                        sqt.rearrange("p j d -> p (j d)"),
                        gth.rearrange("p j d -> p (j d)"))

                    # transposed sums on PE: psum[d, e] = sum_j x / x^2
                    ps_s = ps_st.tile([P, P], f32, tag="ps_s", name="ps_s")
                    ps_q = ps_st.tile([P, P], f32, tag="ps_q", name="ps_q")
                    for j in range(K):
                        nc.tensor.matmul(ps_s, gth[:, j, :], identb,
                                         start=(j == 0), stop=(j == K - 1))
                    ps_l = ps_st.tile([P, P], f32, tag="ps_l", name="ps_l")
                    nc.tensor.matmul(ps_l, gth[:, K - 1, :], identb,
                                     start=True, stop=True)
                    for j in range(K):
                        nc.tensor.matmul(ps_q, sqt[:, j, :], identb,
                                         start=(j == 0), stop=(j == K - 1))
                    ps_lq = ps_st.tile([P, P], f32, tag="ps_lq", name="ps_lq")
                    nc.tensor.matmul(ps_lq, sqt[:, K - 1, :], identb,
                                     start=True, stop=True)

                    # pad corrections: subtract (K-d) * last member (and sq)
                    tmp = tmps.tile([P, P], f32, name="tmp")
                    nc.vector.tensor_tensor(tmp, ps_l, coef_bc[:, cs],
                                            mybir.AluOpType.mult)
                    nc.vector.tensor_tensor(sT[:, cs], ps_s, tmp,
                                            mybir.AluOpType.subtract)
                    tmq = tmps.tile([P, P], f32, name="tmq")
                    nc.vector.tensor_tensor(tmq, ps_lq, coef_bc[:, cs],
                                            mybir.AluOpType.mult)
                    nc.vector.tensor_tensor(qT[:, cs], ps_q, tmq,
                                            mybir.AluOpType.subtract)

                    # max/min over members (edge-major) then PE transpose
                    gv = gth.rearrange("p j d -> p d j")
                    rmax = stage.tile([P, D], f32, tag="rmax", name="rmax")
                    rmin = stage.tile([P, D], f32, tag="rmin", name="rmin")
                    nc.vector.tensor_reduce(rmax, gv, axis=mybir.AxisListType.X,
                                            op=mybir.AluOpType.max)
                    nc.vector.tensor_reduce(rmin, gv, axis=mybir.AxisListType.X,
                                            op=mybir.AluOpType.min)
                    for dst, src in ((maxT, rmax), (minT, rmin)):
                        tps = ps_tr.tile([P, P], f32, tag="tr", name="tr")
                        nc.tensor.transpose(tps, src, ident)
                        nc.any.tensor_copy(out=dst[:, cs], in_=tps)

                # ---- mu / sigma / delta (feature-major) ----
                nc.vector.tensor_tensor(mu, sT, rdeg_bc, mybir.AluOpType.mult)
                nc.vector.tensor_tensor(qT, qT, rdeg_bc, mybir.AluOpType.mult)
                nc.vector.tensor_tensor(sT, mu, mu, mybir.AluOpType.mult)
                nc.vector.tensor_tensor(qT, qT, sT, mybir.AluOpType.subtract)
                nc.vector.tensor_scalar_max(qT, qT, 1e-8)
                nc.scalar.sqrt(sig, qT)
                nc.vector.tensor_tensor(maxT, maxT, minT,
                                        mybir.AluOpType.subtract)
                hT = []
                for b, src_t in ((0, mu), (1, sig), (2, maxT)):
                    hb = mlp.tile([P, MC], bf, tag=f"hb{b}", name=f"hb{b}")
                    nc.any.tensor_copy(out=hb, in_=src_t)
                    hT.append(hb)

                # ---- MLP (feature-major, all 512 edges in one free dim) ----
                l1_ps = ps_mlp.tile([MLP_H, MC], f32, tag="mlp", name="l1_ps")
                for b in range(3):
                    nc.tensor.matmul(l1_ps, w1T_sb[:, b, :], hT[b],
                                     start=(b == 0), stop=(b == 2))
                l1 = mlp.tile([MLP_H, MC], f32, tag="l1s", name="l1")
                nc.scalar.activation(l1, l1_ps,
                                     mybir.ActivationFunctionType.Relu,
                                     bias=b1_sb[:, 0:1], scale=1.0)
                l2_ps = ps_mlp.tile([32, MC], f32, tag="mlp", name="l2_ps")
                nc.tensor.matmul(l2_ps, w2T_sb, l1, start=True, stop=True)
                l2 = mlp.tile([32, MC], f32, tag="l2s", name="l2")
                nc.scalar.activation(l2, l2_ps,
                                     mybir.ActivationFunctionType.Relu,
                                     bias=b2_sb[:, 0:1], scale=1.0)
                l3_ps = ps_mlp.tile([1, MC], f32, tag="mlp", name="l3_ps")
                nc.tensor.matmul(l3_ps, w3T_sb, l2, start=True, stop=True)
                w_sb = mlp.tile([1, MC], f32, tag="w", name="w_sb")
                nc.scalar.activation(w_sb, l3_ps,
                                     mybir.ActivationFunctionType.Sigmoid,
                                     bias=b3_sb[:, 0:1], scale=1.0)
                nc.sync.dma_start(out_d[:, :], w_sb)

    nc.finalize()
    return nc


def kernel(**inputs):
    from concourse import bass2jax

    in_maps, eids, kg = _prep(**inputs)
    nc = _build(kg)
    res = bass2jax.run_bass_via_pjrt(nc, in_maps, n_cores=C)
    out = np.empty(M, np.float32)
    for c in range(C):
        out[eids[c]] = res[c]["out"].reshape(MC)
    return out



# revision 9
# speedup vs baseline: 1.1401x; 1.1401x over previous
"""Trainium2 Bass kernel for DynamicHyperedgeWeightLearner.

Strategy (8 NeuronCores, SPMD single NEFF), v2 — upload-minimal:
  - The graded time is dominated by host->device input bytes, so the
    dense H slab (8 MB/core) and [x|x^2] (4 MB/core) uploads of v1 are
    gone.  Each core receives only: x in bf16 (2 MB), an int16 member-
    index table (~30 KB), per-edge 1/deg and pad-correction scalars,
    and the tiny MLP weights.
  - Hyperedges (M=4096) are degree-sorted and dealt into 8 cores x
    4 groups x 128 edges with a shared per-group pad K (group max
    degree), so every core sees identical shapes (one NEFF for all).
  - Per core, dma_gather (non-transpose; transpose mode faults on this
    HW path; SWDGE ring is hard-capped at 1024 descriptors/call) lands
    member rows edge-major: partition = edge, free axis = member x dim.
    One gather feeds everything:
      max/min  -> DVE reduce over the member axis (pad = last member),
                  then PE transpose to feature-major.
      sum/sumsq-> accumulated directly in transposed (feature-major)
                  form on PE as K matmuls against a bf16 identity;
                  squares from ScalarE (bf16); the pad contribution
                  (K-d)*x_last is subtracted using broadcast per-edge
                  coefficient rows.  Empty edges are padded with node 0
                  and coef K, making all stats 0.
  - The tiny MLP runs feature-major on PE; t_embed is folded into an
    effective b1 on the host (constant across hyperedges).

bf16 note: x quantization changes the final sigmoids by ~1e-4 relative,
far below the 2e-2 gate.
"""

import numpy as np
import ml_dtypes

N, M, D = 8192, 4096, 128
P = 128
C = 8                 # cores
GROUPS = 4            # degree groups
EPC = P               # edges per core per group
MC = GROUPS * EPC     # 512 edges per core
GSZ = C * EPC         # 1024 global edges per group
T_DIM, MLP_H = 32, 64
RING = 1024           # SWDGE descriptor-ring capacity (scratch/16)
CH = RING // P        # member columns per gather call
NQ = 4                # SWDGE queues (ucode max)

last_run_info = {}


def _iw(kg):
    return 8 * sum(kg)


def _prep(node_embeddings, incidence_matrix, time_step,
          W_t, b_t, W1, b1, W2, b2, W3, b3):
    """Host-side preprocessing -> per-core input maps + assembly info."""
    bf16 = ml_dtypes.bfloat16
    x = np.ascontiguousarray(node_embeddings, dtype=np.float32)
    H = np.ascontiguousarray(incidence_matrix, dtype=np.float32)

    nodes, edges = np.nonzero(H)
    order = np.argsort(edges, kind="stable")
    n_sorted = nodes[order].astype(np.int32)
    deg = np.bincount(edges, minlength=M).astype(np.int64)
    offs = np.zeros(M + 1, np.int64)
    np.cumsum(deg, out=offs[1:])

    rank = np.argsort(-deg, kind="stable")      # edge ids by degree desc
    kg = [int(max(1, deg[rank[g * GSZ:(g + 1) * GSZ]].max()))
          for g in range(GROUPS)]

    xg = np.ascontiguousarray(x).astype(bf16)               # (N, D)
    NS = N // C                                             # x shard rows

    # fp32 scalar chain identical to the reference
    t = np.float32(np.asarray(time_step, dtype=np.float32).reshape(()))
    t_embed = np.maximum(
        (t * np.asarray(W_t, np.float32)[:, 0] + np.asarray(b_t, np.float32)),
        np.float32(0.0)).astype(np.float32)
    W1 = np.asarray(W1, np.float32)
    b1_eff = (np.asarray(b1, np.float32)
              + W1[:, 3 * D:] @ t_embed).astype(np.float32)
    w1T = np.ascontiguousarray(W1[:, :3 * D].T)                   # (384, 64)
    w1T_dev = np.ascontiguousarray(
        w1T.reshape(3, P, MLP_H).transpose(1, 0, 2)).astype(bf16)  # (128, 3, 64)
    w2T = np.ascontiguousarray(np.asarray(W2, np.float32).T)      # (64, 32)
    w3T = np.ascontiguousarray(np.asarray(W3, np.float32).T)      # (32, 1)
    b1_dev = b1_eff.reshape(MLP_H, 1).copy()
    b2 = np.asarray(b2, np.float32).reshape(32, 1).copy()
    b3 = np.asarray(b3, np.float32).reshape(1, 1).copy()

    in_maps, eids = [], []
    for c in range(C):
        e = np.concatenate(
            [rank[g * GSZ + c * EPC: g * GSZ + (c + 1) * EPC]
             for g in range(GROUPS)])
        eids.append(e)
        cols = []
        coefP = np.zeros((P, GROUPS), np.float32)
        rdegP = np.zeros((P, GROUPS), np.float32)
        for g in range(GROUPS):
            K = kg[g]
            Jm = np.zeros((K, P), np.int16)
            for p in range(P):
                eid = e[g * EPC + p]
                d0 = int(deg[eid])
                if d0 > 0:
                    mem = n_sorted[offs[eid]:offs[eid] + d0]
                    Jm[:d0, p] = mem
                    Jm[d0:, p] = mem[-1]
                coefP[p, g] = K - d0
                rdegP[p, g] = 1.0 / max(d0, 1)
            lin = Jm.reshape(-1)
            cols.append(lin.reshape(-1, 16).T.copy())     # (16, K*8)
        idx16 = np.ascontiguousarray(np.concatenate(cols, axis=1), np.int16)
        coef = np.ascontiguousarray(coefP)              # (P, GROUPS) edge-major
        rdeg = np.ascontiguousarray(rdegP.T.reshape(1, MC))
        in_maps.append({
            "xg": np.ascontiguousarray(xg[c * NS:(c + 1) * NS]),
            "idx16": idx16, "coef": coef, "rdeg": rdeg,
            "w1T": w1T_dev, "b1": b1_dev,
            "w2T": w2T, "b2": b2, "w3T": w3T, "b3": b3,
        })
    return in_maps, eids, kg


def _build(kg, loops=1, ag=True):
    """Build the SPMD Bass program (one NEFF, all 8 cores).

    ag=False builds a single-core variant with a full (unsharded) x input
    and no collective — used only for local TimelineSim profiling."""
    import concourse.mybir as mybir
    import concourse.tile as tile
    from concourse import bacc
    from concourse.masks import make_identity

    f32 = mybir.dt.float32
    bf = mybir.dt.bfloat16
    iw = _iw(kg)

    nc = bacc.Bacc("TRN2", num_devices=C if ag else None,
                   num_swdge_queues=NQ)
    xg_d = nc.dram_tensor("xg", [N // C if ag else N, D], bf,
                          kind="ExternalInput")
    idx_d = nc.dram_tensor("idx16", [16, iw], mybir.dt.int16,
                           kind="ExternalInput")
    coef_d = nc.dram_tensor("coef", [P, GROUPS], f32, kind="ExternalInput")
    rdeg_d = nc.dram_tensor("rdeg", [1, MC], f32, kind="ExternalInput")
    w1T_d = nc.dram_tensor("w1T", [P, 3, MLP_H], bf, kind="ExternalInput")
    b1_d = nc.dram_tensor("b1", [MLP_H, 1], f32, kind="ExternalInput")
    w2T_d = nc.dram_tensor("w2T", [MLP_H, 32], f32, kind="ExternalInput")
    b2_d = nc.dram_tensor("b2", [32, 1], f32, kind="ExternalInput")
    w3T_d = nc.dram_tensor("w3T", [32, 1], f32, kind="ExternalInput")
    b3_d = nc.dram_tensor("b3", [1, 1], f32, kind="ExternalInput")
    out_d = nc.dram_tensor("out", [1, MC], f32, kind="ExternalOutput")

    with tile.TileContext(nc) as tc:
        with (
            tc.tile_pool(name="dram", bufs=1, space="DRAM") as dram,
            tc.tile_pool(name="singles", bufs=1) as singles,
            tc.tile_pool(name="gpool", bufs=3) as gpool,
            tc.tile_pool(name="sqpool", bufs=3) as sqpool,
            tc.tile_pool(name="stage", bufs=4) as stage,
            tc.tile_pool(name="fold", bufs=2) as fold,
            tc.tile_pool(name="tmps", bufs=2) as tmps,
            tc.tile_pool(name="mlp", bufs=1) as mlp,
            tc.tile_pool(name="ps_st", bufs=2, space="PSUM") as ps_st,
            tc.tile_pool(name="ps_tr", bufs=2, space="PSUM") as ps_tr,
            tc.tile_pool(name="ps_mlp", bufs=1, space="PSUM") as ps_mlp,
        ):
            # ---- x shards -> full x via on-device AllGather ----
            if ag:
                ag_in = dram.tile([N // C, D], bf)
                xfull = dram.tile([N, D], bf, addr_space="Shared")
                nc.gpsimd.dma_start(ag_in[:, :], xg_d[:, :])
                nc.gpsimd.collective_compute(
                    "AllGather", mybir.AluOpType.bypass,
                    replica_groups=[list(range(C))],
                    ins=[ag_in[:, :].opt()], outs=[xfull[:, :].opt()],
                )
            else:
                xfull = xg_d

            # ---- resident loads ----
            idx_sb = singles.tile([P, iw], mybir.dt.int16)
            for r in range(8):
                nc.sync.dma_start(idx_sb[16 * r:16 * (r + 1), :], idx_d[:, :])
            coefE = singles.tile([P, GROUPS], f32)
            nc.sync.dma_start(coefE, coef_d[:, :])
            rdeg_bc = singles.tile([P, MC], f32)
            nc.gpsimd.dma_start(rdeg_bc, rdeg_d[0:1, :].to_broadcast((P, MC)))
            w1T_sb = singles.tile([P, 3, MLP_H], bf)
            nc.sync.dma_start(w1T_sb, w1T_d[:, :, :])
            b1_sb = singles.tile([MLP_H, 1], f32)
            nc.sync.dma_start(b1_sb, b1_d[:, :])
            w2T_sb = singles.tile([MLP_H, 32], f32)
            nc.sync.dma_start(w2T_sb, w2T_d[:, :])
            b2_sb = singles.tile([32, 1], f32)
            nc.sync.dma_start(b2_sb, b2_d[:, :])
            w3T_sb = singles.tile([32, 1], f32)
            nc.sync.dma_start(w3T_sb, w3T_d[:, :])
            b3_sb = singles.tile([1, 1], f32)
            nc.sync.dma_start(b3_sb, b3_d[:, :])

            ident = singles.tile([P, P], f32)
            make_identity(nc, ident)
            identb = singles.tile([P, P], bf)
            make_identity(nc, identb)

            Kmax = max(kg)
            import os as _os
            _no_gather = bool(int(_os.environ.get("TIME_NO_GATHER", "0")))

            qrr = [0]  # SWDGE queue round-robin counter

            pre = None
            if _no_gather:
                pre = []
                offp = 0
                for g in range(GROUPS):
                    K = kg[g]
                    gf = singles.tile([P, Kmax, D], bf, name=f"pregth{g}")
                    for j0 in range(0, K, CH):
                        ch = min(CH, K - j0)
                        nc.gpsimd.dma_gather(
                            gf[:, j0:j0 + ch, :], xfull[:, :],
                            idx_sb[:, offp + 8 * j0:offp + 8 * (j0 + ch)],
                            ch * P, ch * P, D,
                            queue_num=qrr[0] % NQ,
                        )
                        qrr[0] += 1
                    offp += 8 * K
                    pre.append(gf)

            for _rep in range(loops):
                maxT = mlp.tile([P, MC], f32, tag="maxT", name="maxT")
                minT = mlp.tile([P, MC], f32, tag="minT", name="minT")
                sT = mlp.tile([P, MC], f32, tag="sT", name="sT")
                qT = mlp.tile([P, MC], f32, tag="qT", name="qT")
                mu = mlp.tile([P, MC], f32, tag="mu", name="mu")
                sig = mlp.tile([P, MC], f32, tag="sig", name="sig")

                off = 0
                for g in range(GROUPS):
                    K = kg[g]
                    cs = slice(g * P, (g + 1) * P)
                    if _no_gather:
                        gfull = pre[g]
                    else:
                        gfull = gpool.tile([P, Kmax, D], bf, name="gth")
                    gth = gfull[:, 0:K, :]
                    if not _no_gather:
                        for j0 in range(0, K, CH):
                            ch = min(CH, K - j0)
                            nc.gpsimd.dma_gather(
                                gth[:, j0:j0 + ch, :], xfull[:, :],
                                idx_sb[:, off + 8 * j0:off + 8 * (j0 + ch)],
                                ch * P, ch * P, D,
                                queue_num=qrr[0] % NQ,
                            )
                            qrr[0] += 1
                    off += 8 * K
                    sfull = sqpool.tile([P, Kmax, D], bf, name="sq")
                    sqt = sfull[:, 0:K, :]
                    nc.scalar.square(
                        sqt.rearrange("p j d -> p (j d)"),
                        gth.rearrange("p j d -> p (j d)"))

                    # edge-major sums on PE: identity stationary, moving 4
                    # member slots per Matmult, out AP j-stride 0 so PSUM
                    # accumulates the colliding column writes (HW-verified).
                    ps_se = ps_st.tile([P, P], f32, tag="ps_s", name="ps_se")
                    ps_qe = ps_st.tile([P, P], f32, tag="ps_q", name="ps_qe")
                    for dst, src_t in ((ps_se, gth), (ps_qe, sqt)):
                        for c0 in range(0, K, 4):
                            cw = min(4, K - c0)
                            rep = dst[:, :].unsqueeze(1).to_broadcast((P, cw, D))
                            nc.tensor.matmul(rep, identb, src_t[:, c0:c0 + cw, :],
                                             start=(c0 == 0),
                                             stop=(c0 + cw == K))

                    # pad corrections edge-major: coefE[:, g] is a per-edge
                    # ([p,1]) scalar; last member row is an SBUF view.
                    sE = tmps.tile([P, P], f32, name="sE")
                    qE = tmps.tile([P, P], f32, name="qE")
                    tmp = tmps.tile([P, P], f32, name="tmp")
                    nc.vector.tensor_scalar_mul(tmp, gth[:, K - 1, :],
                                                coefE[:, g:g + 1])
                    nc.vector.tensor_tensor(sE, ps_se, tmp,
                                            mybir.AluOpType.subtract)
                    tmq = tmps.tile([P, P], f32, name="tmq")
                    nc.vector.tensor_scalar_mul(tmq, sqt[:, K - 1, :],
                                                coefE[:, g:g + 1])
                    nc.vector.tensor_tensor(qE, ps_qe, tmq,
                                            mybir.AluOpType.subtract)

                    # max/min: one contiguous bf16 fold (overlap-safe for
                    # idempotent ops), then a half-size strided reduce.
                    h = (K + 1) // 2
                    fmax = fold.tile([P, (Kmax + 1) // 2, D], bf, name="fmax")
                    fmin = fold.tile([P, (Kmax + 1) // 2, D], bf, name="fmin")
                    nc.vector.tensor_tensor(fmax[:, 0:h, :], gth[:, 0:h, :],
                                            gth[:, K - h:K, :],
                                            mybir.AluOpType.max)
                    nc.vector.tensor_tensor(fmin[:, 0:h, :], gth[:, 0:h, :],
                                            gth[:, K - h:K, :],
                                            mybir.AluOpType.min)
                    rmax = stage.tile([P, D], f32, tag="rmax", name="rmax")
                    rmin = stage.tile([P, D], f32, tag="rmin", name="rmin")
                    nc.vector.tensor_reduce(
                        rmax, fmax[:, 0:h, :].rearrange("p j d -> p d j"),
                        axis=mybir.AxisListType.X, op=mybir.AluOpType.max)
                    nc.vector.tensor_reduce(
                        rmin, fmin[:, 0:h, :].rearrange("p j d -> p d j"),
                        axis=mybir.AxisListType.X, op=mybir.AluOpType.min)

                    # transpose the four edge-major stats to feature-major
                    for dst, src in ((sT, sE), (qT, qE),
                                     (maxT, rmax), (minT, rmin)):
                        tps = ps_tr.tile([P, P], f32, tag="tr", name="tr")
                        nc.tensor.transpose(tps, src, ident)
                        nc.any.tensor_copy(out=dst[:, cs], in_=tps)

                # ---- mu / sigma / delta (feature-major) ----
                nc.vector.tensor_tensor(mu, sT, rdeg_bc, mybir.AluOpType.mult)
                nc.vector.tensor_tensor(qT, qT, rdeg_bc, mybir.AluOpType.mult)
                nc.vector.tensor_tensor(sT, mu, mu, mybir.AluOpType.mult)
                nc.vector.tensor_tensor(qT, qT, sT, mybir.AluOpType.subtract)
                nc.vector.tensor_scalar_max(qT, qT, 1e-8)
                nc.scalar.sqrt(sig, qT)
                nc.vector.tensor_tensor(maxT, maxT, minT,
                                        mybir.AluOpType.subtract)
                hT = []
                for b, src_t in ((0, mu), (1, sig), (2, maxT)):
                    hb = mlp.tile([P, MC], bf, tag=f"hb{b}", name=f"hb{b}")
                    nc.any.tensor_copy(out=hb, in_=src_t)
                    hT.append(hb)

                # ---- MLP (feature-major, all 512 edges in one free dim) ----
                l1_ps = ps_mlp.tile([MLP_H, MC], f32, tag="mlp", name="l1_ps")
                for b in range(3):
                    nc.tensor.matmul(l1_ps, w1T_sb[:, b, :], hT[b],
                                     start=(b == 0), stop=(b == 2))
                l1 = mlp.tile([MLP_H, MC], f32, tag="l1s", name="l1")
                nc.scalar.activation(l1, l1_ps,
                                     mybir.ActivationFunctionType.Relu,
                                     bias=b1_sb[:, 0:1], scale=1.0)
                l2_ps = ps_mlp.tile([32, MC], f32, tag="mlp", name="l2_ps")
                nc.tensor.matmul(l2_ps, w2T_sb, l1, start=True, stop=True)
                l2 = mlp.tile([32, MC], f32, tag="l2s", name="l2")
                nc.scalar.activation(l2, l2_ps,
                                     mybir.ActivationFunctionType.Relu,
                                     bias=b2_sb[:, 0:1], scale=1.0)
                l3_ps = ps_mlp.tile([1, MC], f32, tag="mlp", name="l3_ps")
                nc.tensor.matmul(l3_ps, w3T_sb, l2, start=True, stop=True)
                w_sb = mlp.tile([1, MC], f32, tag="w", name="w_sb")
                nc.scalar.activation(w_sb, l3_ps,
                                     mybir.ActivationFunctionType.Sigmoid,
                                     bias=b3_sb[:, 0:1], scale=1.0)
                nc.sync.dma_start(out_d[:, :], w_sb)

    nc.finalize()
    return nc


def kernel(**inputs):
    from concourse import bass2jax

    in_maps, eids, kg = _prep(**inputs)
    nc = _build(kg)
    res = bass2jax.run_bass_via_pjrt(nc, in_maps, n_cores=C)
    out = np.empty(M, np.float32)
    for c in range(C):
        out[eids[c]] = res[c]["out"].reshape(MC)
    return out

